# revision 4
# baseline (speedup 1.0000x reference)
"""Log2-level hardware-constrained quantizer for Trainium2 (Bass/Tile).

Math: with levels [-8,-4,-2,-1,0,1,2,4,8] and weights clipped to [-1,1],
only levels {-1, 0, 1} can ever be nearest, and the argmin tie-breaks
(first-min) resolve to:
    out = 0.125 if w >  0.5
    out = 0     if -0.5 < w <= 0.5
    out = -0.125 if w <= -0.5
(the /MAX_LEVEL*WEIGHT_MAX scale is 1/8; near-zero snap is a no-op).

Implemented per 128xFD tile as:
    t1 = (w is_gt  0.5) * 0.125     # {0, +0.125}
    t2 = (w is_le -0.5) * -0.125    # {0, -0.125}
    o  = t1 + t2
each a single DVE instruction; the whole kernel is memory-bound.
"""

import numpy as np

import concourse.bacc as bacc
import concourse.mybir as mybir
from concourse.bass_utils import run_bass_kernel_spmd
from concourse.tile import TileContext

N_CORES = 8
ROWS, COLS = 4096, 8192
ROWS_PER_CORE = ROWS // N_CORES  # 512
P = 128
FD = 4096  # free-dim tile width (2 MiB f32 tiles)

_nc_cache = None


def _build_nc():
    global _nc_cache
    if _nc_cache is not None:
        return _nc_cache

    # Bacc (not raw Bass): its compile pipeline runs generate_event_semaphores,
    # which splits multi-sem waits to satisfy TRN2's 1-wait-per-instruction
    # limit — raw Bass modules fail walrus codegen with "Too many sync wait
    # commands".
    nc = bacc.Bacc("TRN2")
    f32 = mybir.dt.float32
    w = nc.dram_tensor("weights", [ROWS_PER_CORE, COLS], f32, kind="ExternalInput")
    o = nc.dram_tensor("out", [ROWS_PER_CORE, COLS], f32, kind="ExternalOutput")

    n_row_tiles = ROWS_PER_CORE // P  # 4
    n_col_tiles = COLS // FD          # 2

    with TileContext(nc) as tc:
        with (
            tc.tile_pool(name="w", bufs=3) as wp,
            tc.tile_pool(name="t1", bufs=2) as t1p,
            tc.tile_pool(name="t2", bufs=2) as t2p,
            tc.tile_pool(name="o", bufs=3) as outp,
        ):
            for i in range(n_row_tiles):
                for j in range(n_col_tiles):
                    rs, cs = i * P, j * FD
                    wt = wp.tile([P, FD], f32)
                    nc.sync.dma_start(out=wt[:], in_=w[rs : rs + P, cs : cs + FD])
                    t1 = t1p.tile([P, FD], f32)
                    nc.vector.tensor_scalar(
                        out=t1[:], in0=wt[:], scalar1=0.5, scalar2=0.125,
                        op0=mybir.AluOpType.is_gt, op1=mybir.AluOpType.mult,
                    )
                    t2 = t2p.tile([P, FD], f32)
                    nc.vector.tensor_scalar(
                        out=t2[:], in0=wt[:], scalar1=-0.5, scalar2=-0.125,
                        op0=mybir.AluOpType.is_le, op1=mybir.AluOpType.mult,
                    )
                    ot = outp.tile([P, FD], f32)
                    nc.vector.tensor_tensor(
                        out=ot[:], in0=t1[:], in1=t2[:], op=mybir.AluOpType.add,
                    )
                    nc.sync.dma_start(out=o[rs : rs + P, cs : cs + FD], in_=ot[:])

    nc.finalize()
    _nc_cache = nc
    return nc


def _run(weights: np.ndarray, **spmd_kwargs):
    nc = _build_nc()
    weights = np.ascontiguousarray(np.asarray(weights, dtype=np.float32))
    assert weights.shape == (ROWS, COLS), weights.shape
    shards = np.split(weights, N_CORES, axis=0)
    in_maps = [{"weights": s} for s in shards]
    res = run_bass_kernel_spmd(nc, in_maps, core_ids=list(range(N_CORES)), **spmd_kwargs)
    out = np.concatenate([r["out"] for r in res.results], axis=0)
    return out, res


def kernel(weights: np.ndarray) -> np.ndarray:
    out, _ = _run(weights)
    return out


# revision 5
# speedup vs baseline: 1004.4265x; 1004.4265x over previous
"""Log2-level hardware-constrained quantizer for Trainium2 (Bass/Tile).

Math: with levels [-8,-4,-2,-1,0,1,2,4,8] and weights clipped to [-1,1],
only levels {-1, 0, 1} can ever be nearest, and the argmin tie-breaks
(first-min) resolve to:
    out = 0.125 if w >  0.5
    out = 0     if -0.5 < w <= 0.5
    out = -0.125 if w <= -0.5
(the /MAX_LEVEL*WEIGHT_MAX scale is 1/8; near-zero snap is a no-op).

Implemented per 128xFD tile as:
    t1 = (w is_gt  0.5) * 0.125     # {0, +0.125}
    t2 = (w is_le -0.5) * -0.125    # {0, -0.125}
    o  = t1 + t2
each a single DVE instruction; the whole kernel is memory-bound.
"""

import numpy as np

import concourse.bacc as bacc
import concourse.mybir as mybir
from concourse.bass_utils import run_bass_kernel_spmd
from concourse.tile import TileContext

N_CORES = 8
ROWS, COLS = 4096, 8192
ROWS_PER_CORE = ROWS // N_CORES  # 512
P = 128
FD = 4096  # free-dim tile width (2 MiB f32 tiles)

_nc_cache = None


def _build_nc():
    global _nc_cache
    if _nc_cache is not None:
        return _nc_cache

    # Bacc (not raw Bass): its compile pipeline runs generate_event_semaphores,
    # which splits multi-sem waits to satisfy TRN2's 1-wait-per-instruction
    # limit — raw Bass modules fail walrus codegen with "Too many sync wait
    # commands".
    nc = bacc.Bacc("TRN2")
    f32 = mybir.dt.float32
    w = nc.dram_tensor("weights", [ROWS_PER_CORE, COLS], f32, kind="ExternalInput")
    o = nc.dram_tensor("out", [ROWS_PER_CORE, COLS], f32, kind="ExternalOutput")

    # Flat per-partition-contiguous view: partition p owns a contiguous 128 KiB
    # run of the shard, so every DMA descriptor is a 16 KiB contiguous burst.
    wf = w.rearrange("(p a) k -> p (a k)", p=P)  # [128, 32768]
    of = o.rearrange("(p a) k -> p (a k)", p=P)
    n_tiles = wf.shape[1] // FD  # 8

    with TileContext(nc) as tc:
        with (
            tc.tile_pool(name="w", bufs=3) as wp,
            tc.tile_pool(name="t1", bufs=2) as t1p,
            tc.tile_pool(name="t2", bufs=2) as t2p,
            tc.tile_pool(name="o", bufs=3) as outp,
        ):
            for j in range(n_tiles):
                cs = j * FD
                wt = wp.tile([P, FD], f32)
                # loads on SP HWDGE; stores on Activation HWDGE — separate
                # queue sets overlap better than funnelling both through SP
                # (TimelineSim: 96.6 us vs 101.5 us one-shot).
                nc.sync.dma_start(out=wt[:], in_=wf[:, cs : cs + FD])
                t1 = t1p.tile([P, FD], f32)
                nc.vector.tensor_scalar(
                    out=t1[:], in0=wt[:], scalar1=0.5, scalar2=0.125,
                    op0=mybir.AluOpType.is_gt, op1=mybir.AluOpType.mult,
                )
                t2 = t2p.tile([P, FD], f32)
                nc.vector.tensor_scalar(
                    out=t2[:], in0=wt[:], scalar1=-0.5, scalar2=-0.125,
                    op0=mybir.AluOpType.is_le, op1=mybir.AluOpType.mult,
                )
                ot = outp.tile([P, FD], f32)
                nc.vector.tensor_tensor(
                    out=ot[:], in0=t1[:], in1=t2[:], op=mybir.AluOpType.add,
                )
                nc.scalar.dma_start(out=of[:, cs : cs + FD], in_=ot[:])

    nc.finalize()
    _nc_cache = nc
    return nc


def _run(weights: np.ndarray, **spmd_kwargs):
    nc = _build_nc()
    weights = np.ascontiguousarray(np.asarray(weights, dtype=np.float32))
    assert weights.shape == (ROWS, COLS), weights.shape
    shards = np.split(weights, N_CORES, axis=0)
    in_maps = [{"weights": s} for s in shards]
    res = run_bass_kernel_spmd(nc, in_maps, core_ids=list(range(N_CORES)), **spmd_kwargs)
    out = np.concatenate([r["out"] for r in res.results], axis=0)
    return out, res


def kernel(weights: np.ndarray) -> np.ndarray:
    out, _ = _run(weights)
    return out
